# revision 27
# baseline (speedup 1.0000x reference)
"""Distributed Trainium2 Bass kernel for causal multi-head attention with RoPE.

Problem: B=2, T=2048, C=1024, H=16 heads, D=64. 8 NeuronCores.

Sharding (2x4 grid): core c handles batch b = c//4 and the 4 heads
g = c%4 -> heads [4g..4g+4). QKV projections + RoPE + causal attention run
fully locally per core in a "transposed" layout (qT/kT = [D_heads, T]) so
no on-chip transposes are ever needed:

  qT = Wq_slice.T @ x.T            (lhsT = Wq natural, rhs = x.T)
  scoresT[k,q] = kT.T-block @ qT   (softmax along PARTITION axis)
  outT = [v|1].T @ exp(scoresT)    (ones column yields softmax denominators)
  outW = Wo_cols.T @ attn_allT     (attn stays transposed through Wo)

v2: fused software pipeline. The QKV+RoPE work for token-chunk s+1 is cut
into 8 "slots" (~8 matmuls each) that are interleaved between the causal
attention blocks of q-chunk s, so the tensor engine fills the bubbles that
the exp-bound softmax leaves, and the scalar engine (exp) never waits for a
monolithic projection phase. Wo matmuls for qc 0..2 are likewise
interleaved into the last attention chunk. PSUM: 2 banks slot pool +
2 banks scores + 4 banks AV accumulators = 8.
"""

import numpy as np
import ml_dtypes

import concourse.bacc as bacc
import concourse.mybir as mybir
import concourse.tile as tile
from concourse.bass_utils import run_bass_kernel_spmd

B, T, C, H, D = 2, 2048, 1024, 16, 64
NCORES = 8
HPC = 4              # heads per core
CPC = HPC * D        # channels per core (256)
NPAIR = 2            # head pairs per core
QC = 4               # q-chunks of 512
KB = T // 128        # k-blocks of 128
CCH = C // 128       # contraction chunks of 128
F32 = mybir.dt.float32
BF16 = mybir.dt.bfloat16
AF = mybir.ActivationFunctionType
RGROUPS = [[0, 1, 2, 3], [4, 5, 6, 7]]

_cache = {}


def _build_nc():
    nc = bacc.Bacc(None, target_bir_lowering=False, debug=False, num_devices=NCORES)

    xT = nc.declare_dram_parameter("xT", [C, T], BF16, isOutput=False)
    wq = nc.declare_dram_parameter("wq", [C, CPC], BF16, isOutput=False)
    wk = nc.declare_dram_parameter("wk", [C, CPC], BF16, isOutput=False)
    wv = nc.declare_dram_parameter("wv", [C, CPC], BF16, isOutput=False)
    wo = nc.declare_dram_parameter("wo", [C, CPC], BF16, isOutput=False)
    cosP = nc.declare_dram_parameter("cosP", [128, T], BF16, isOutput=False)
    sinP = nc.declare_dram_parameter("sinP", [128, T], BF16, isOutput=False)
    utst = nc.declare_dram_parameter("utst", [128, 128], BF16, isOutput=False)
    dneg = nc.declare_dram_parameter("dneg", [128, 128], BF16, isOutput=False)
    smat = nc.declare_dram_parameter("smat", [128, 128], BF16, isOutput=False)
    out = nc.declare_dram_parameter("out", [CPC, T], F32, isOutput=True)

    with tile.TileContext(nc) as tc:
        with (
            tc.tile_pool(name="resident", bufs=1) as rp,
            tc.tile_pool(name="rope", bufs=3) as ropep,
            tc.tile_pool(name="expp", bufs=10) as expp,
            tc.tile_pool(name="outb", bufs=4) as outbp,
            tc.tile_pool(name="agsb", bufs=32) as agp,
            tc.tile_pool(name="small", bufs=3) as smp,
            tc.tile_pool(name="aps", bufs=2, space="PSUM") as aps,
            tc.tile_pool(name="scp", bufs=2, space="PSUM") as scp,
            tc.tile_pool(name="avp", bufs=1, space="PSUM") as avp,
            tc.tile_pool(name="dram", bufs=1, space="DRAM") as dram,
        ):
            # ---------------- resident SBUF ----------------
            xbf = rp.tile([128, CCH * T], BF16)          # x.T in [nch][cc] blocks
            wqbf = rp.tile([128, CCH * CPC], BF16)
            wkbf = rp.tile([128, CCH * CPC], BF16)
            wvbf = rp.tile([128, CCH * CPC], BF16)
            wobf = rp.tile([128, CCH * CPC], BF16)
            cos_sb = rp.tile([128, T], BF16)
            sin_sb = rp.tile([128, T], BF16)
            utst_sb = rp.tile([128, 128], BF16)
            dneg_sb = rp.tile([128, 128], BF16)
            smat_bf = rp.tile([128, 128], BF16)
            ones_sb = rp.tile([1, 64], BF16)
            qTbf = rp.tile([128, NPAIR * T], BF16)       # rope'd qT, per pair
            kTbf = rp.tile([128, NPAIR * T], BF16)
            vsb = rp.tile([128, HPC * KB * 65], BF16)    # [v | 1] per head per k-block

            # ---------------- loads: 3 DMA queues, consumption order ----------
            # sync: wq, x0[4:8], x1, wo    scalar: x0[0:4], wv, x2
            # gpsimd: warmup-AG, cos, smat, wk, sin, mask, vsb-ones, x3
            warm_in = dram.tile([1, 64], BF16, tag="warmin", name="warm_in")
            warm_out = dram.tile([4, 64], BF16, tag="warmout", name="warm_out")
            nc.gpsimd.collective_compute(
                "AllGather", mybir.AluOpType.bypass, replica_groups=RGROUPS,
                ins=[warm_in.opt()], outs=[warm_out.opt()],
            )
            nc.gpsimd.memset(ones_sb[:], 1.0)
            for cc in range(CCH):
                nc.sync.dma_start(
                    wqbf[:, cc * CPC:(cc + 1) * CPC], wq[cc * 128:(cc + 1) * 128, :])
            nc.gpsimd.dma_start(cos_sb[:], cosP[:])
            nc.gpsimd.dma_start(smat_bf[:], smat[:])

            def xload(eng, nch, cc):
                eng.dma_start(
                    xbf[:, (nch * CCH + cc) * 512:(nch * CCH + cc + 1) * 512],
                    xT[cc * 128:(cc + 1) * 128, nch * 512:(nch + 1) * 512])

            for cc in range(4):
                xload(nc.scalar, 0, cc)
            for cc in range(4, CCH):
                xload(nc.sync, 0, cc)
            for cc in range(CCH):
                nc.gpsimd.dma_start(
                    wkbf[:, cc * CPC:(cc + 1) * CPC], wk[cc * 128:(cc + 1) * 128, :])
            nc.gpsimd.dma_start(sin_sb[:], sinP[:])
            nc.gpsimd.dma_start(utst_sb[:], utst[:])
            nc.gpsimd.dma_start(dneg_sb[:], dneg[:])
            for cc in range(CCH):
                nc.scalar.dma_start(
                    wvbf[:, cc * CPC:(cc + 1) * CPC], wv[cc * 128:(cc + 1) * 128, :])
            nc.gpsimd.memset(vsb[:], 1.0)
            for cc in range(CCH):
                xload(nc.sync, 1, cc)
            for cc in range(CCH):
                xload(nc.scalar, 2, cc)
            for cc in range(CCH):
                xload(nc.gpsimd, 3, cc)
            for cc in range(CCH):
                nc.sync.dma_start(
                    wobf[:, cc * CPC:(cc + 1) * CPC], wo[cc * 128:(cc + 1) * 128, :])

            # ---------------- A slots: QKV + RoPE for one 512-token chunk -----
            # slot = (S1, S2); S1 at block j, S2 after AV of block j+1.
            def qk_slot(nch, w_sb, t_sb, p):
                st = {}
                nsl = slice(nch * 512, nch * 512 + 512)

                def s1():
                    ps = aps.tile([128, 512], F32, tag="a",
                                  name=f"ps{nch}_{0 if w_sb is wqbf else 1}_{p}")
                    for cc in range(CCH):
                        nc.tensor.matmul(
                            ps[:],
                            w_sb[:, cc * CPC + p * 128: cc * CPC + (p + 1) * 128],
                            xbf[:, (nch * CCH + cc) * 512:(nch * CCH + cc + 1) * 512],
                            start=(cc == 0), stop=(cc == CCH - 1),
                        )
                    qub = ropep.tile([128, 512], BF16, tag="qub")
                    nc.vector.tensor_copy(qub[:], ps[:])
                    st["ps"] = ps
                    st["qub"] = qub

                def s2():
                    rot = aps.tile([128, 512], F32, tag="a",
                                   name=f"rot{nch}_{0 if w_sb is wqbf else 1}_{p}")
                    nc.tensor.matmul(rot[:], smat_bf[:], st["qub"][:], start=True, stop=True)
                    rotb = ropep.tile([128, 512], BF16, tag="rotb")
                    nc.vector.tensor_copy(rotb[:], rot[:])
                    t1 = ropep.tile([128, 512], BF16, tag="t1")
                    nc.vector.tensor_mul(t1[:], st["qub"][:], cos_sb[:, nsl])
                    t2 = ropep.tile([128, 512], BF16, tag="t2")
                    nc.vector.tensor_mul(t2[:], rotb[:], sin_sb[:, nsl])
                    nc.vector.tensor_add(
                        t_sb[:, p * T + nch * 512: p * T + nch * 512 + 512], t1[:], t2[:])

                return (s1, s2)

            def v_slot(tch):
                nch = tch // 4

                def s1():
                    ps_v = aps.tile([128, 512], F32, tag="a", name=f"psv{tch}")
                    for cc in range(CCH):
                        nc.tensor.matmul(
                            ps_v[:, 0:CPC],
                            xbf[:, (nch * CCH + cc) * 512 + (tch % 4) * 128:
                                (nch * CCH + cc) * 512 + (tch % 4) * 128 + 128],
                            wvbf[:, cc * CPC:(cc + 1) * CPC],
                            start=(cc == 0), stop=(cc == CCH - 1),
                        )
                    # one strided copy: [128, 4 heads, 64] -> vsb[h*KB*65 + tch*65 ..+64]
                    dst = vsb[:].rearrange("p (h x) -> p h x", h=HPC)[
                        :, :, tch * 65: tch * 65 + 64]
                    src = ps_v[:, 0:CPC].rearrange("p (h d) -> p h d", h=HPC)
                    nc.scalar.copy(dst, src)

                return (s1, None)

            def a_slots(nch):
                return [
                    qk_slot(nch, wqbf, qTbf, 0),
                    qk_slot(nch, wkbf, kTbf, 0),
                    qk_slot(nch, wqbf, qTbf, 1),
                    qk_slot(nch, wkbf, kTbf, 1),
                    v_slot(4 * nch + 0),
                    v_slot(4 * nch + 1),
                    v_slot(4 * nch + 2),
                    v_slot(4 * nch + 3),
                ]

            # ---------------- AllGather / Wo plumbing -------------------------
            ag_outs = []
            bands = [dram.tile([CPC, 512], BF16, tag=f"agin{i}", name=f"band{i}")
                     for i in range(QC - 1)]
            bands3 = [dram.tile([128, 512], BF16, tag=f"agin3{p}", name=f"band3{p}")
                      for p in range(NPAIR)]
            _ag_sb = {}

            def wo_srcs(qc):
                if qc == 3:
                    order = [0, 2, 4, 6, 1, 3, 5, 7]
                    srcs = {cc: (ag_outs[3 + cc % 2], (cc // 2) * 128) for cc in range(CCH)}
                else:
                    order = list(range(CCH))
                    srcs = {cc: (ag_outs[qc], cc * 128) for cc in range(CCH)}
                return order, srcs

            def emit_wo_loads(qc):
                if (qc, 0) in _ag_sb:
                    return
                order, srcs = wo_srcs(qc)
                for cc in order:
                    src, row = srcs[cc]
                    t = agp.tile([128, 512], BF16, name=f"ag_{qc}_{cc}", tag="ag")
                    nc.gpsimd.dma_start(t[:], src[row:row + 128, :])
                    _ag_sb[(qc, cc)] = t

            def wo_slot(qc, mch):
                st = {}

                def s1():
                    emit_wo_loads(qc)
                    order, srcs = wo_srcs(qc)
                    ps_o = aps.tile([128, 512], F32, tag="a", name=f"pso{qc}_{mch}")
                    for idx, cc in enumerate(order):
                        nc.tensor.matmul(
                            ps_o[:],
                            wobf[:, cc * CPC + mch * 128: cc * CPC + (mch + 1) * 128],
                            _ag_sb[(qc, cc)][:],
                            start=(idx == 0), stop=(idx == CCH - 1),
                        )
                    st["ps"] = ps_o

                def s2():
                    osb = outbp.tile([128, 512], F32, tag="osb")
                    nc.vector.tensor_copy(osb[:], st["ps"][:])
                    nc.sync.dma_start(
                        out[mch * 128:(mch + 1) * 128, qc * 512:(qc + 1) * 512], osb[:])

                return (s1, s2)

            # ---------------- normalize flush --------------------------------
            _recs = {}

            def make_normalize(qc, p, av):
                def flush():
                    # broadcast the (bf16) denominators, then ONE full-width
                    # reciprocal for both heads (DVE cost ~ free dim only)
                    ps_b = aps.tile([128, 512], F32, tag="a", name=f"psb{qc}_{p}")
                    for i in range(2):
                        nc.tensor.matmul(ps_b[i * 64:(i + 1) * 64, :], ones_sb[:],
                                         _recs[(qc, p, i)][:],
                                         start=True, stop=True, tile_position=(0, i * 64))
                    bden = smp.tile([128, 512], F32, tag="bden", name=f"bden{qc}_{p}")
                    nc.vector.tensor_copy(bden[:], ps_b[:])
                    bc = smp.tile([128, 512], F32, tag="bcs")
                    nc.vector.reciprocal_approx_fast(bc[:], bden[:])
                    ob = outbp.tile([128, 512], BF16, tag="ob", name=f"ob{qc}_{p}")
                    for i in range(2):
                        nc.vector.tensor_mul(ob[i * 64:(i + 1) * 64, :], av[i][0:64, :],
                                             bc[i * 64:(i + 1) * 64, :])
                    if qc == 3:
                        nc.sync.dma_start(bands3[p][:], ob[:])
                        ag_out = dram.tile([4 * 128, 512], BF16, tag=f"agout3{p}",
                                           name=f"agout3{p}")
                        nc.gpsimd.collective_compute(
                            "AllGather", mybir.AluOpType.bypass,
                            replica_groups=RGROUPS,
                            ins=[bands3[p].opt()], outs=[ag_out.opt()],
                        )
                        ag_outs.append(ag_out)
                    else:
                        nc.sync.dma_start(bands[qc][p * 128:(p + 1) * 128, :], ob[:])
                        if p == NPAIR - 1:
                            ag_out = dram.tile([4 * CPC, 512], BF16, tag=f"agout{qc}",
                                               name=f"agout{qc}")
                            nc.gpsimd.collective_compute(
                                "AllGather", mybir.AluOpType.bypass,
                                replica_groups=RGROUPS,
                                ins=[bands[qc].opt()], outs=[ag_out.opt()],
                            )
                            ag_outs.append(ag_out)
                return flush

            # ---------------- fused B(s) + interleaved slots ------------------
            def run_B(s, slots):
                nonlocal pending
                j = 0          # linear block index within this B
                prev_s2 = None
                pend_av = None  # AV matmuls staggered one block behind scores
                for p in range(NPAIR):
                    av = [avp.tile([65, 512], F32, tag=f"av{i}", name=f"av{s}_{p}_{i}")
                          for i in range(2)]
                    kmax = 4 * s + 4
                    for kb in range(kmax):
                        nqs = max(s * 512, kb * 128)
                        noff = nqs - s * 512
                        n = 512 - noff
                        # right-aligned: head0 at [512-n:512], head1 at [512:512+n]
                        # -> exp covers exactly 2n contiguous cols, bank-legal.
                        # Causal mask accumulated on PE (-240 strict-upper) so
                        # AV depends only on exp, never on the DVE queue.
                        diag = nqs == kb * 128
                        ps_s = scp.tile([128, 1024], F32, tag="s", name=f"pss{s}_{p}_{kb}")
                        for i in range(2):
                            hs = slice(i * 64, (i + 1) * 64)
                            base = 512 - n if i == 0 else 512
                            nc.tensor.matmul(
                                ps_s[:, base: base + n],
                                kTbf[hs, p * T + kb * 128: p * T + kb * 128 + 128],
                                qTbf[hs, p * T + nqs: p * T + nqs + n],
                                start=True, stop=not diag,
                                tile_position=(i * 64, 0),
                            )
                        if diag:
                            for i in range(2):
                                base = 512 - n if i == 0 else 512
                                nc.tensor.matmul(
                                    ps_s[:, base: base + 128],
                                    dneg_sb[:], utst_sb[:],
                                    start=False, stop=True,
                                )
                        e = expp.tile([128, 1024], BF16, tag="e", name=f"e{s}_{p}_{kb}")
                        nc.scalar.activation(e[:, 512 - n:512 + n], ps_s[:, 512 - n:512 + n],
                                             AF.Exp, scale=0.125)
                        if pend_av is not None:
                            pend_av()

                        def make_av(av=av, e=e, p=p, kb=kb, kmax=kmax, n=n, noff=noff):
                            def emit():
                                for i in range(2):
                                    h = 2 * p + i
                                    vbase = h * KB * 65 + kb * 65
                                    base = 512 - n if i == 0 else 512
                                    nc.tensor.matmul(
                                        av[i][:, noff:512],
                                        vsb[:, vbase: vbase + 65],
                                        e[:, base: base + n],
                                        start=(kb == 0), stop=(kb == kmax - 1),
                                    )
                            return emit

                        pend_av = make_av()
                        slot = slots.get(j)
                        if slot is not None:
                            slot[0]()
                        if prev_s2 is not None:
                            prev_s2()
                        prev_s2 = slot[1] if slot is not None else None
                        if kb == 0 and pending is not None:
                            pending()
                            pending = None
                        j += 1
                    pend_av()
                    pend_av = None
                    # denominator rows -> SBUF bf16 (cast in the copy)
                    for i in range(2):
                        dnb = smp.tile([1, 512], BF16, tag=f"rec{i}", name=f"dnb{s}_{p}_{i}")
                        nc.vector.tensor_copy(dnb[:], av[i][64:65, :])
                        _recs[(s, p, i)] = dnb
                    if pending is not None:
                        pending()
                    pending = make_normalize(s, p, av)
                    if p == NPAIR - 1:
                        # flush now: the AllGather trigger leaves ~10us earlier
                        pending()
                        pending = None
                if prev_s2 is not None:
                    prev_s2()

            # ---------------- emit the whole pipeline -------------------------
            pending = None

            # A(0) standalone (ramp)
            slots0 = a_slots(0)
            prev = None
            for j in range(len(slots0) + 1):
                if j < len(slots0):
                    slots0[j][0]()
                if prev is not None:
                    prev()
                prev = slots0[j][1] if j < len(slots0) else None

            run_B(0, dict(enumerate(a_slots(1))))
            run_B(1, dict(enumerate(a_slots(2))))
            run_B(2, dict(enumerate(a_slots(3))))
            # C(0..2) interleaved into B(3), placed so each AG is long done
            run_B(3, {2: wo_slot(0, 0), 4: wo_slot(0, 1),
                      8: wo_slot(1, 0), 10: wo_slot(1, 1),
                      24: wo_slot(2, 0), 26: wo_slot(2, 1)})

            # tail: flush(3, p1) then Wo for qc=3
            if pending is not None:
                pending()
                pending = None
            for mch in range(2):
                s1, s2 = wo_slot(3, mch)
                s1()
                s2()
    return nc


def _get_nc():
    if "nc" not in _cache:
        nc = _build_nc()
        nc.finalize()
        _cache["nc"] = nc
    return _cache["nc"]


def _host_tables(freqs_cos, freqs_sin):
    cosP = np.empty((128, T), np.float32)
    sinP = np.empty((128, T), np.float32)
    for r in range(128):
        i = (r % 64) // 2
        cosP[r] = freqs_cos[:, i]
        sinP[r] = freqs_sin[:, i]
    # utst[k, q] = 1 where k > q (strictly future key); dneg = diag(-240)
    utst = np.tril(np.ones((128, 128), np.float32), -1)
    dneg = np.diag(np.full(128, -240.0, np.float32))
    smat = np.zeros((128, 128), np.float32)
    for i in range(64):
        smat[2 * i + 1, 2 * i] = -1.0   # rot[2i] = -q[2i+1]
        smat[2 * i, 2 * i + 1] = 1.0    # rot[2i+1] = +q[2i]
    return cosP, sinP, utst, dneg, smat


def _install_trace_hooks():
    import sys, types
    try:
        import antenv.axon_hooks  # noqa: F401
        return True
    except ImportError:
        pass
    try:
        from trn_agent_boot.trn_boot import _ntff_profile_via_ctypes
        mod = types.ModuleType("antenv.axon_hooks")
        mod._hook = _ntff_profile_via_ctypes("/opt/axon/libaxon_pjrt.so")
        mod.set_axon_ntff_profile_hook = lambda h: setattr(mod, "_hook", h)
        mod.get_axon_ntff_profile_hook = lambda: mod._hook
        sys.modules["antenv.axon_hooks"] = mod
        import antenv
        antenv.axon_hooks = mod
        import concourse.bass_utils as bu
        bu.upload_artifacts = lambda tmpdir: f"file://{tmpdir}"
        return True
    except Exception:
        return False


def _bf16(a):
    return np.ascontiguousarray(a).astype(ml_dtypes.bfloat16)


def kernel(x, freqs_cos, freqs_sin, Wq, Wk, Wv, Wo, _trace=False):
    x = np.asarray(x, np.float32)
    freqs_cos = np.asarray(freqs_cos, np.float32)
    freqs_sin = np.asarray(freqs_sin, np.float32)
    Wq, Wk, Wv, Wo = (np.asarray(w, np.float32) for w in (Wq, Wk, Wv, Wo))
    cosP, sinP, utst, dneg, smat = _host_tables(freqs_cos, freqs_sin)

    in_maps = []
    for c in range(NCORES):
        b, g = c // 4, c % 4
        sl = slice(g * CPC, (g + 1) * CPC)
        in_maps.append({
            "xT": _bf16(x[b].T),
            "wq": _bf16(Wq[:, sl]),
            "wk": _bf16(Wk[:, sl]),
            "wv": _bf16(Wv[:, sl]),
            "wo": _bf16(Wo[:, sl]),
            "cosP": _bf16(cosP), "sinP": _bf16(sinP),
            "utst": _bf16(utst), "dneg": _bf16(dneg), "smat": _bf16(smat),
        })

    nc = _get_nc()
    if _trace:
        _trace = _install_trace_hooks()
    res = run_bass_kernel_spmd(nc, in_maps, core_ids=list(range(NCORES)), trace=_trace)
    _cache["last_res"] = res

    out = np.empty((B, T, C), np.float32)
    for c in range(NCORES):
        b, g = c // 4, c % 4
        out[b][:, g * CPC:(g + 1) * CPC] = res.results[c]["out"].T
    return out


# revision 29
# speedup vs baseline: 1.2617x; 1.2617x over previous
"""Distributed Trainium2 Bass kernel for causal multi-head attention with RoPE.

Problem: B=2, T=2048, C=1024, H=16 heads, D=64. 8 NeuronCores.

Sharding (2x4 grid): core c handles batch b = c//4 and the 4 heads
g = c%4 -> heads [4g..4g+4). QKV projections + RoPE + causal attention run
fully locally per core in a "transposed" layout (qT/kT = [D_heads, T]) so
no on-chip transposes are ever needed:

  qT = Wq_slice.T @ x.T            (lhsT = Wq natural, rhs = x.T)
  scoresT[k,q] = kT.T-block @ qT   (softmax along PARTITION axis)
  outT = [v|1].T @ exp(scoresT)    (ones column yields softmax denominators)
  outW = Wo_cols.T @ attn_allT     (attn stays transposed through Wo)

v2: fused software pipeline. The QKV+RoPE work for token-chunk s+1 is cut
into 8 "slots" (~8 matmuls each) that are interleaved between the causal
attention blocks of q-chunk s, so the tensor engine fills the bubbles that
the exp-bound softmax leaves, and the scalar engine (exp) never waits for a
monolithic projection phase. Wo matmuls for qc 0..2 are likewise
interleaved into the last attention chunk. PSUM: 2 banks slot pool +
2 banks scores + 4 banks AV accumulators = 8.
"""

import numpy as np
import ml_dtypes

import concourse.bacc as bacc
import concourse.mybir as mybir
import concourse.tile as tile
from concourse.bass_utils import run_bass_kernel_spmd

B, T, C, H, D = 2, 2048, 1024, 16, 64
NCORES = 8
HPC = 4              # heads per core
CPC = HPC * D        # channels per core (256)
NPAIR = 2            # head pairs per core
QC = 4               # q-chunks of 512
KB = T // 128        # k-blocks of 128
CCH = C // 128       # contraction chunks of 128
F32 = mybir.dt.float32
BF16 = mybir.dt.bfloat16
AF = mybir.ActivationFunctionType
RGROUPS = [[0, 1, 2, 3], [4, 5, 6, 7]]

_cache = {}


def _build_nc():
    nc = bacc.Bacc(None, target_bir_lowering=False, debug=False, num_devices=NCORES)

    xT = nc.declare_dram_parameter("xT", [C, T], BF16, isOutput=False)
    wq = nc.declare_dram_parameter("wq", [C, CPC], BF16, isOutput=False)
    wk = nc.declare_dram_parameter("wk", [C, CPC], BF16, isOutput=False)
    wv = nc.declare_dram_parameter("wv", [C, CPC], BF16, isOutput=False)
    wo = nc.declare_dram_parameter("wo", [C, CPC], BF16, isOutput=False)
    cosP = nc.declare_dram_parameter("cosP", [128, T], BF16, isOutput=False)
    sinP = nc.declare_dram_parameter("sinP", [128, T], BF16, isOutput=False)
    utst = nc.declare_dram_parameter("utst", [128, 128], BF16, isOutput=False)
    dneg = nc.declare_dram_parameter("dneg", [128, 128], BF16, isOutput=False)
    smat = nc.declare_dram_parameter("smat", [128, 128], BF16, isOutput=False)
    out = nc.declare_dram_parameter("out", [CPC, T], F32, isOutput=True)

    with tile.TileContext(nc) as tc:
        with (
            tc.tile_pool(name="resident", bufs=1) as rp,
            tc.tile_pool(name="rope", bufs=3) as ropep,
            tc.tile_pool(name="expp", bufs=10) as expp,
            tc.tile_pool(name="outb", bufs=4) as outbp,
            tc.tile_pool(name="agsb", bufs=32) as agp,
            tc.tile_pool(name="small", bufs=3) as smp,
            tc.tile_pool(name="aps", bufs=2, space="PSUM") as aps,
            tc.tile_pool(name="scp", bufs=2, space="PSUM") as scp,
            tc.tile_pool(name="avp", bufs=1, space="PSUM") as avp,
            tc.tile_pool(name="dram", bufs=1, space="DRAM") as dram,
        ):
            # ---------------- resident SBUF ----------------
            xbf = rp.tile([128, CCH * T], BF16)          # x.T in [nch][cc] blocks
            wqbf = rp.tile([128, CCH * CPC], BF16)
            wkbf = rp.tile([128, CCH * CPC], BF16)
            wvbf = rp.tile([128, CCH * CPC], BF16)
            wobf = rp.tile([128, CCH * CPC], BF16)
            cos_sb = rp.tile([128, T], BF16)
            sin_sb = rp.tile([128, T], BF16)
            utst_sb = rp.tile([128, 128], BF16)
            dneg_sb = rp.tile([128, 128], BF16)
            smat_bf = rp.tile([128, 128], BF16)
            ones_sb = rp.tile([1, 64], BF16)
            qTbf = rp.tile([128, NPAIR * T], BF16)       # rope'd qT, per pair
            kTbf = rp.tile([128, NPAIR * T], BF16)
            vsb = rp.tile([128, HPC * KB * 65], BF16)    # [v | 1] per head per k-block

            # ---------------- loads: 3 DMA queues, consumption order ----------
            # sync: wq, x0[4:8], x1, wo    scalar: x0[0:4], wv, x2
            # gpsimd: warmup-AG, cos, smat, wk, sin, mask, vsb-ones, x3
            warm_in = dram.tile([1, 64], BF16, tag="warmin", name="warm_in")
            warm_out = dram.tile([4, 64], BF16, tag="warmout", name="warm_out")
            nc.gpsimd.collective_compute(
                "AllGather", mybir.AluOpType.bypass, replica_groups=RGROUPS,
                ins=[warm_in.opt()], outs=[warm_out.opt()],
            )
            nc.gpsimd.memset(ones_sb[:], 1.0)
            for cc in range(CCH):
                nc.sync.dma_start(
                    wqbf[:, cc * CPC:(cc + 1) * CPC], wq[cc * 128:(cc + 1) * 128, :])
            nc.gpsimd.dma_start(cos_sb[:], cosP[:])
            nc.gpsimd.dma_start(smat_bf[:], smat[:])

            def xload(eng, nch, cc):
                eng.dma_start(
                    xbf[:, (nch * CCH + cc) * 512:(nch * CCH + cc + 1) * 512],
                    xT[cc * 128:(cc + 1) * 128, nch * 512:(nch + 1) * 512])

            for cc in range(4):
                xload(nc.scalar, 0, cc)
            for cc in range(4, CCH):
                xload(nc.sync, 0, cc)
            for cc in range(CCH):
                nc.gpsimd.dma_start(
                    wkbf[:, cc * CPC:(cc + 1) * CPC], wk[cc * 128:(cc + 1) * 128, :])
            nc.gpsimd.dma_start(sin_sb[:], sinP[:])
            nc.gpsimd.dma_start(utst_sb[:], utst[:])
            nc.gpsimd.dma_start(dneg_sb[:], dneg[:])
            for cc in range(CCH):
                nc.scalar.dma_start(
                    wvbf[:, cc * CPC:(cc + 1) * CPC], wv[cc * 128:(cc + 1) * 128, :])
            nc.gpsimd.memset(vsb[:], 1.0)
            for cc in range(CCH):
                xload(nc.sync, 1, cc)
            for cc in range(CCH):
                xload(nc.scalar, 2, cc)
            for cc in range(CCH):
                xload(nc.gpsimd, 3, cc)
            for cc in range(CCH):
                nc.sync.dma_start(
                    wobf[:, cc * CPC:(cc + 1) * CPC], wo[cc * 128:(cc + 1) * 128, :])

            # ---------------- A slots: QKV + RoPE for one 512-token chunk -----
            # slot = (S1, S2); S1 at block j, S2 after AV of block j+1.
            def qk_slot(nch, w_sb, t_sb, p):
                st = {}
                nsl = slice(nch * 512, nch * 512 + 512)

                def s1():
                    ps = aps.tile([128, 512], F32, tag="a",
                                  name=f"ps{nch}_{0 if w_sb is wqbf else 1}_{p}")
                    for cc in range(CCH):
                        nc.tensor.matmul(
                            ps[:],
                            w_sb[:, cc * CPC + p * 128: cc * CPC + (p + 1) * 128],
                            xbf[:, (nch * CCH + cc) * 512:(nch * CCH + cc + 1) * 512],
                            start=(cc == 0), stop=(cc == CCH - 1),
                        )
                    qub = ropep.tile([128, 512], BF16, tag="qub")
                    nc.vector.tensor_copy(qub[:], ps[:])
                    st["ps"] = ps
                    st["qub"] = qub

                def s2():
                    rot = aps.tile([128, 512], F32, tag="a",
                                   name=f"rot{nch}_{0 if w_sb is wqbf else 1}_{p}")
                    nc.tensor.matmul(rot[:], smat_bf[:], st["qub"][:], start=True, stop=True)
                    rotb = ropep.tile([128, 512], BF16, tag="rotb")
                    nc.vector.tensor_copy(rotb[:], rot[:])
                    t1 = ropep.tile([128, 512], BF16, tag="t1")
                    nc.vector.tensor_mul(t1[:], st["qub"][:], cos_sb[:, nsl])
                    t2 = ropep.tile([128, 512], BF16, tag="t2")
                    nc.vector.tensor_mul(t2[:], rotb[:], sin_sb[:, nsl])
                    nc.vector.tensor_add(
                        t_sb[:, p * T + nch * 512: p * T + nch * 512 + 512], t1[:], t2[:])

                return (s1, s2)

            def v_slot(tch):
                nch = tch // 4

                def s1():
                    ps_v = aps.tile([128, 512], F32, tag="a", name=f"psv{tch}")
                    for cc in range(CCH):
                        nc.tensor.matmul(
                            ps_v[:, 0:CPC],
                            xbf[:, (nch * CCH + cc) * 512 + (tch % 4) * 128:
                                (nch * CCH + cc) * 512 + (tch % 4) * 128 + 128],
                            wvbf[:, cc * CPC:(cc + 1) * CPC],
                            start=(cc == 0), stop=(cc == CCH - 1),
                        )
                    # one strided copy: [128, 4 heads, 64] -> vsb[h*KB*65 + tch*65 ..+64]
                    dst = vsb[:].rearrange("p (h x) -> p h x", h=HPC)[
                        :, :, tch * 65: tch * 65 + 64]
                    src = ps_v[:, 0:CPC].rearrange("p (h d) -> p h d", h=HPC)
                    nc.scalar.copy(dst, src)

                return (s1, None)

            def a_slots(nch):
                return [
                    qk_slot(nch, wqbf, qTbf, 0),
                    qk_slot(nch, wkbf, kTbf, 0),
                    qk_slot(nch, wqbf, qTbf, 1),
                    qk_slot(nch, wkbf, kTbf, 1),
                    v_slot(4 * nch + 0),
                    v_slot(4 * nch + 1),
                    v_slot(4 * nch + 2),
                    v_slot(4 * nch + 3),
                ]

            # ---------------- AllGather / Wo plumbing -------------------------
            ag_outs = []
            bands = [dram.tile([CPC, 512], BF16, tag=f"agin{i}", name=f"band{i}")
                     for i in range(QC - 1)]
            bands3 = [dram.tile([128, 512], BF16, tag=f"agin3{p}", name=f"band3{p}")
                      for p in range(NPAIR)]
            _ag_sb = {}

            def wo_srcs(qc):
                if qc == 3:
                    order = [0, 2, 4, 6, 1, 3, 5, 7]
                    srcs = {cc: (ag_outs[3 + cc % 2], (cc // 2) * 128) for cc in range(CCH)}
                else:
                    order = list(range(CCH))
                    srcs = {cc: (ag_outs[qc], cc * 128) for cc in range(CCH)}
                return order, srcs

            def emit_wo_loads(qc):
                if (qc, 0) in _ag_sb:
                    return
                order, srcs = wo_srcs(qc)
                for cc in order:
                    src, row = srcs[cc]
                    t = agp.tile([128, 512], BF16, name=f"ag_{qc}_{cc}", tag="ag")
                    nc.gpsimd.dma_start(t[:], src[row:row + 128, :])
                    _ag_sb[(qc, cc)] = t

            def wo_slot(qc, mch):
                st = {}

                def s1():
                    emit_wo_loads(qc)
                    order, srcs = wo_srcs(qc)
                    ps_o = aps.tile([128, 512], F32, tag="a", name=f"pso{qc}_{mch}")
                    for idx, cc in enumerate(order):
                        nc.tensor.matmul(
                            ps_o[:],
                            wobf[:, cc * CPC + mch * 128: cc * CPC + (mch + 1) * 128],
                            _ag_sb[(qc, cc)][:],
                            start=(idx == 0), stop=(idx == CCH - 1),
                        )
                    st["ps"] = ps_o

                def s2():
                    osb = outbp.tile([128, 512], F32, tag="osb")
                    nc.vector.tensor_copy(osb[:], st["ps"][:])
                    nc.sync.dma_start(
                        out[mch * 128:(mch + 1) * 128, qc * 512:(qc + 1) * 512], osb[:])

                return (s1, s2)

            # ---------------- normalize flush --------------------------------
            _recs = {}

            def make_normalize(qc, p, av):
                def flush():
                    # broadcast the (bf16) denominators, then ONE full-width
                    # reciprocal for both heads (DVE cost ~ free dim only)
                    ps_b = aps.tile([128, 512], F32, tag="a", name=f"psb{qc}_{p}")
                    for i in range(2):
                        nc.tensor.matmul(ps_b[i * 64:(i + 1) * 64, :], ones_sb[:],
                                         _recs[(qc, p, i)][:],
                                         start=True, stop=True, tile_position=(0, i * 64))
                    bden = smp.tile([128, 512], F32, tag="bden", name=f"bden{qc}_{p}")
                    nc.vector.tensor_copy(bden[:], ps_b[:])
                    bc = smp.tile([128, 512], F32, tag="bcs")
                    nc.vector.reciprocal_approx_fast(bc[:], bden[:])
                    ob = outbp.tile([128, 512], BF16, tag="ob", name=f"ob{qc}_{p}")
                    for i in range(2):
                        nc.vector.tensor_mul(ob[i * 64:(i + 1) * 64, :], av[i][0:64, :],
                                             bc[i * 64:(i + 1) * 64, :])
                    if qc == 3:
                        nc.sync.dma_start(bands3[p][:], ob[:])
                        ag_out = dram.tile([4 * 128, 512], BF16, tag=f"agout3{p}",
                                           name=f"agout3{p}")
                        nc.gpsimd.collective_compute(
                            "AllGather", mybir.AluOpType.bypass,
                            replica_groups=RGROUPS,
                            ins=[bands3[p].opt()], outs=[ag_out.opt()],
                        )
                        ag_outs.append(ag_out)
                    else:
                        nc.sync.dma_start(bands[qc][p * 128:(p + 1) * 128, :], ob[:])
                        if p == NPAIR - 1:
                            ag_out = dram.tile([4 * CPC, 512], BF16, tag=f"agout{qc}",
                                               name=f"agout{qc}")
                            nc.gpsimd.collective_compute(
                                "AllGather", mybir.AluOpType.bypass,
                                replica_groups=RGROUPS,
                                ins=[bands[qc].opt()], outs=[ag_out.opt()],
                            )
                            ag_outs.append(ag_out)
                return flush

            # ---------------- fused B(s) + interleaved slots ------------------
            def run_B(s, slots):
                nonlocal pending
                j = 0          # linear block index within this B
                prev_s2 = None
                pend_av = None  # AV matmuls staggered one block behind scores
                for p in range(NPAIR):
                    av = [avp.tile([65, 512], F32, tag=f"av{i}", name=f"av{s}_{p}_{i}")
                          for i in range(2)]
                    kmax = 4 * s + 4
                    for kb in range(kmax):
                        nqs = max(s * 512, kb * 128)
                        noff = nqs - s * 512
                        n = 512 - noff
                        # right-aligned: head0 at [512-n:512], head1 at [512:512+n]
                        # -> exp covers exactly 2n contiguous cols, bank-legal.
                        # Causal mask accumulated on PE (-240 strict-upper) so
                        # AV depends only on exp, never on the DVE queue.
                        diag = nqs == kb * 128
                        ps_s = scp.tile([128, 1024], F32, tag="s", name=f"pss{s}_{p}_{kb}")
                        for i in range(2):
                            hs = slice(i * 64, (i + 1) * 64)
                            base = 512 - n if i == 0 else 512
                            nc.tensor.matmul(
                                ps_s[:, base: base + n],
                                kTbf[hs, p * T + kb * 128: p * T + kb * 128 + 128],
                                qTbf[hs, p * T + nqs: p * T + nqs + n],
                                start=True, stop=not diag,
                                tile_position=(i * 64, 0),
                            )
                        if diag:
                            for i in range(2):
                                base = 512 - n if i == 0 else 512
                                nc.tensor.matmul(
                                    ps_s[:, base: base + 128],
                                    dneg_sb[:], utst_sb[:],
                                    start=False, stop=True,
                                )
                        e = expp.tile([128, 1024], BF16, tag="e", name=f"e{s}_{p}_{kb}")
                        nc.scalar.activation(e[:, 512 - n:512 + n], ps_s[:, 512 - n:512 + n],
                                             AF.Exp, scale=0.125)
                        if pend_av is not None:
                            pend_av()

                        def make_av(av=av, e=e, p=p, kb=kb, kmax=kmax, n=n, noff=noff):
                            def emit():
                                for i in range(2):
                                    h = 2 * p + i
                                    vbase = h * KB * 65 + kb * 65
                                    base = 512 - n if i == 0 else 512
                                    nc.tensor.matmul(
                                        av[i][:, noff:512],
                                        vsb[:, vbase: vbase + 65],
                                        e[:, base: base + n],
                                        start=(kb == 0), stop=(kb == kmax - 1),
                                    )
                            return emit

                        pend_av = make_av()
                        slot = slots.get(j)
                        if slot is not None:
                            slot[0]()
                        if prev_s2 is not None:
                            prev_s2()
                        prev_s2 = slot[1] if slot is not None else None
                        if kb == 0 and pending is not None:
                            pending()
                            pending = None
                        j += 1
                    pend_av()
                    pend_av = None
                    # denominator rows -> SBUF bf16 (cast in the copy)
                    for i in range(2):
                        dnb = smp.tile([1, 512], BF16, tag=f"rec{i}", name=f"dnb{s}_{p}_{i}")
                        nc.vector.tensor_copy(dnb[:], av[i][64:65, :])
                        _recs[(s, p, i)] = dnb
                    if pending is not None:
                        pending()
                    pending = make_normalize(s, p, av)
                if prev_s2 is not None:
                    prev_s2()

            # ---------------- emit the whole pipeline -------------------------
            pending = None

            # A(0) standalone (ramp)
            slots0 = a_slots(0)
            prev = None
            for j in range(len(slots0) + 1):
                if j < len(slots0):
                    slots0[j][0]()
                if prev is not None:
                    prev()
                prev = slots0[j][1] if j < len(slots0) else None

            run_B(0, dict(enumerate(a_slots(1))))
            run_B(1, dict(enumerate(a_slots(2))))
            run_B(2, dict(enumerate(a_slots(3))))
            # C(0..2) interleaved into B(3), placed so each AG is long done
            run_B(3, {4: wo_slot(0, 0), 6: wo_slot(0, 1),
                      9: wo_slot(1, 0), 11: wo_slot(1, 1),
                      20: wo_slot(2, 0), 22: wo_slot(2, 1)})

            # tail: flush(3, p1) then Wo for qc=3
            if pending is not None:
                pending()
                pending = None
            for mch in range(2):
                s1, s2 = wo_slot(3, mch)
                s1()
                s2()
    return nc


def _get_nc():
    if "nc" not in _cache:
        nc = _build_nc()
        nc.finalize()
        _cache["nc"] = nc
    return _cache["nc"]


def _host_tables(freqs_cos, freqs_sin):
    cosP = np.empty((128, T), np.float32)
    sinP = np.empty((128, T), np.float32)
    for r in range(128):
        i = (r % 64) // 2
        cosP[r] = freqs_cos[:, i]
        sinP[r] = freqs_sin[:, i]
    # utst[k, q] = 1 where k > q (strictly future key); dneg = diag(-240)
    utst = np.tril(np.ones((128, 128), np.float32), -1)
    dneg = np.diag(np.full(128, -240.0, np.float32))
    smat = np.zeros((128, 128), np.float32)
    for i in range(64):
        smat[2 * i + 1, 2 * i] = -1.0   # rot[2i] = -q[2i+1]
        smat[2 * i, 2 * i + 1] = 1.0    # rot[2i+1] = +q[2i]
    return cosP, sinP, utst, dneg, smat


def _install_trace_hooks():
    import sys, types
    try:
        import antenv.axon_hooks  # noqa: F401
        return True
    except ImportError:
        pass
    try:
        from trn_agent_boot.trn_boot import _ntff_profile_via_ctypes
        mod = types.ModuleType("antenv.axon_hooks")
        mod._hook = _ntff_profile_via_ctypes("/opt/axon/libaxon_pjrt.so")
        mod.set_axon_ntff_profile_hook = lambda h: setattr(mod, "_hook", h)
        mod.get_axon_ntff_profile_hook = lambda: mod._hook
        sys.modules["antenv.axon_hooks"] = mod
        import antenv
        antenv.axon_hooks = mod
        import concourse.bass_utils as bu
        bu.upload_artifacts = lambda tmpdir: f"file://{tmpdir}"
        return True
    except Exception:
        return False


def _bf16(a):
    return np.ascontiguousarray(a).astype(ml_dtypes.bfloat16)


def kernel(x, freqs_cos, freqs_sin, Wq, Wk, Wv, Wo, _trace=False):
    x = np.asarray(x, np.float32)
    freqs_cos = np.asarray(freqs_cos, np.float32)
    freqs_sin = np.asarray(freqs_sin, np.float32)
    Wq, Wk, Wv, Wo = (np.asarray(w, np.float32) for w in (Wq, Wk, Wv, Wo))
    cosP, sinP, utst, dneg, smat = _host_tables(freqs_cos, freqs_sin)

    in_maps = []
    for c in range(NCORES):
        b, g = c // 4, c % 4
        sl = slice(g * CPC, (g + 1) * CPC)
        in_maps.append({
            "xT": _bf16(x[b].T),
            "wq": _bf16(Wq[:, sl]),
            "wk": _bf16(Wk[:, sl]),
            "wv": _bf16(Wv[:, sl]),
            "wo": _bf16(Wo[:, sl]),
            "cosP": _bf16(cosP), "sinP": _bf16(sinP),
            "utst": _bf16(utst), "dneg": _bf16(dneg), "smat": _bf16(smat),
        })

    nc = _get_nc()
    if _trace:
        _trace = _install_trace_hooks()
    res = run_bass_kernel_spmd(nc, in_maps, core_ids=list(range(NCORES)), trace=_trace)
    _cache["last_res"] = res

    out = np.empty((B, T, C), np.float32)
    for c in range(NCORES):
        b, g = c // 4, c % 4
        out[b][:, g * CPC:(g + 1) * CPC] = res.results[c]["out"].T
    return out
